# revision 1
# baseline (speedup 1.0000x reference)
"""Trainium2 Bass kernel for nn_Autotuner_FFN (dense MLP, 8-core data parallel).

Fast-path structure (be1=be2=0, bc2=0 — true for this model's inputs):
  * Host precomputes the feature matrix XT [256, B] in fp16: one-hot
    encodings, the 57 sign(x)*ln(|x|+1) transformed features, a ones row
    carrying the folded first-layer bias, zero padding to 2 full K=128
    tiles. LayerNorm affine g is folded into W1/W2 columns (stats use a
    per-partition 1/g prescale); mean-centering is folded into weights.
  * Per 512-sample chunk the device does only:
      L1: 16 fp16 matmuls -> G1 (PSUM)
      LN1: Act Square(G1)->fp8 pairs, DVE relu(G1)->f16 R1 (pv1 DEFERRED)
      stats1: 4 fp8 DoubleRow ones-reduce matmuls; pv1 = AbsRsqrt LUT
      L2: 64 fp16 matmuls over unnormalized R1 -> G2
      LN2: same; variance rescaled by pv1^2 in [1,512] smalls
      L3: 8 fp16 matmuls -> g3; y = pv1*pv2*g3 (+b3)
    LayerNorm scale-invariance makes the deferral exact: relu(c*x) =
    c*relu(x) for c>0, so per-column scales commute out to the end.
  * No PSUM->SBUF copies, no rsqrt broadcast matmuls, no bias adds, no
    device transcendentals except the one AbsRsqrt LUT per LN.
  * All matmul K-tiles are padded to 128 (K<128 matmuls run ~2x slower).
Legacy general path (arbitrary affine/bias) retained below.
"""
import numpy as np

import concourse.bass as bass
import concourse.tile as tile
from concourse import bacc, mybir
from concourse.bass_utils import run_bass_kernel_spmd

AF = mybir.ActivationFunctionType
ALU = mybir.AluOpType
F32 = mybir.dt.float32
F16 = mybir.dt.float16
F8 = mybir.dt.float8e4
DRM = mybir.MatmulPerfMode.DoubleRow
F32R = mybir.dt.float16  # legacy alias

B = 65536
N_CORES = 8
B_CORE = B // N_CORES          # 8192
CH = 512                       # batch chunk (one PSUM bank wide)
NCH = B_CORE // CH             # 16
HID = 1024
MT = HID // 128                # 8 hidden m-tiles
KA, KC = 128, 57               # legacy feature K tiles
EPS = 1e-5
LN2 = float(np.log(2.0))


# ---------------------------------------------------------------- host folds
def _fold_weights(inp):
    f8 = lambda x: np.asarray(x, np.float64)
    W1 = f8(inp["W1"]); b1 = f8(inp["b1"])
    emb_kc = f8(inp["emb_kc"]); emb_nl = f8(inp["emb_nl"])
    op_W = f8(inp["op_W"]); op_b = f8(inp["op_b"])
    emb_c = f8(inp["emb_contig"]); emb_s = f8(inp["emb_scalar"])
    emb_i = f8(inp["emb_indirect"])
    H = W1.shape[1]
    rows_A = []
    bias = b1.copy()
    rows_A.append(emb_kc @ W1[0:16])
    rows_A.append(emb_nl @ W1[16:32])
    W1_op = W1[32:944].reshape(57, 16, H)
    rows_A.append(np.einsum("ij,ijh->ih", op_W, W1_op))
    bias += np.einsum("ij,ijh->h", op_b, W1_op)
    rd_f2, rd_bool, rd_ss = [], [], []
    wd_f2, wd_bool, wd_ss = [], [], []
    for base, f2l, booll, ssl in ((947, rd_f2, rd_bool, rd_ss),
                                  (1027, wd_f2, wd_bool, wd_ss)):
        for d in range(4):
            Wd = W1[base + 20 * d: base + 20 * d + 20]
            f2l.append(Wd[0:2])
            ssl.append(Wd[2:8] / LN2)
            rows_b = []
            for e, sl in ((emb_c, slice(8, 12)), (emb_s, slice(12, 16)),
                          (emb_i, slice(16, 20))):
                rows_b.append((e[1] - e[0]) @ Wd[sl])
                bias += e[0] @ Wd[sl]
            booll.append(np.stack(rows_b))
    rows_A += [np.concatenate(rd_f2), np.concatenate(rd_bool),
               np.concatenate(wd_f2), np.concatenate(wd_bool),
               W1[1110:1112]]
    A = np.concatenate(rows_A)
    C = np.concatenate([W1[944:947] / LN2, W1[1107:1110] / LN2,
                        W1[1112:1115] / LN2,
                        np.concatenate(rd_ss), np.concatenate(wd_ss)])
    W1_eff = np.concatenate([A, np.zeros((3, H)), C])       # [185, H]
    W1c = W1_eff - W1_eff.mean(axis=1, keepdims=True)
    bc1 = bias - bias.mean()
    W2 = f8(inp["W2"]); b2 = f8(inp["b2"])
    W2c = W2 - W2.mean(axis=1, keepdims=True)
    bc2 = b2 - b2.mean()
    return (W1c.astype(np.float32), bc1.astype(np.float32),
            W2c.astype(np.float32), bc2.astype(np.float32))


def _build_xt_raw(inp):
    """[185, B] float32 feature matrix, 57 transform rows still raw."""
    Bn = inp["op_vec"].shape[0]
    kc = np.asarray(inp["kernel_category_idx"]).astype(np.int64)
    nl = np.asarray(inp["num_of_loops_idx"]).astype(np.int64)
    f = lambda k: np.asarray(inp[k], np.float32)
    XT = np.zeros((185, Bn), np.float32)
    XT[0:10] = (np.arange(10)[:, None] == kc[None, :])
    XT[10:26] = (np.arange(16)[:, None] == nl[None, :])
    XT[26:83] = f("op_vec").T
    XT[83:91] = f("read_dep_float")[:, :, 0:2].reshape(Bn, 8).T
    XT[91:103] = np.asarray(inp["read_dep_bools"]).reshape(Bn, 12).T
    XT[103:111] = f("write_dep_float")[:, :, 0:2].reshape(Bn, 8).T
    XT[111:123] = np.asarray(inp["write_dep_bools"]).reshape(Bn, 12).T
    XT[123:125] = f("rest_vec")[:, 3:5].T
    XT[128:131] = f("size_hints").T
    XT[131:137] = f("rest_vec")[:, [0, 1, 2, 5, 6, 7]].T
    XT[137:161] = f("read_dep_float")[:, :, 2:8].reshape(Bn, 24).T
    XT[161:185] = f("write_dep_float")[:, :, 2:8].reshape(Bn, 24).T
    return XT


def _pack128(v):
    """[1024] -> [128, 8] with v[m*128+p] at [p, m]."""
    return np.ascontiguousarray(np.asarray(v, np.float32).reshape(8, 128).T)


# ---------------------------------------------------------------- fast device
FAST_CFG = dict(xin_bufs=3, r_bufs=2, sq_bufs=2, sm_bufs=3,
                ps_mm_bufs=3, ps_st_bufs=2, ps_p3_bufs=2,
                relu_split=0, stats_late=True)


def build_fast(has_b3, loop_iters=None, cfg=None):
    """Fast-path program. has_b3: include final bias add."""
    cfg = {**FAST_CFG, **(cfg or {})}
    nc = bacc.Bacc("TRN2", target_bir_lowering=False, debug=False)
    xt = nc.dram_tensor("xt", [256, B_CORE], F16, kind="ExternalInput")
    w1 = nc.dram_tensor("w1", [256, HID], F16, kind="ExternalInput")
    w2 = nc.dram_tensor("w2", [HID, HID], F16, kind="ExternalInput")
    w3p = nc.dram_tensor("w3p", [128, MT], F16, kind="ExternalInput")
    s1p = nc.dram_tensor("s1p", [128, MT], F32, kind="ExternalInput")
    s2p = nc.dram_tensor("s2p", [128, MT], F32, kind="ExternalInput")
    b3t = nc.dram_tensor("b3t", [1, 1], F32, kind="ExternalInput")
    y = nc.dram_tensor("y", [1, B_CORE], F32, kind="ExternalOutput")

    from contextlib import ExitStack
    with tile.TileContext(nc) as tc, ExitStack() as ctx, \
            nc.allow_low_precision(reason="fp16/fp8 rounding is intentional"):
        const = ctx.enter_context(tc.tile_pool(name="const", bufs=1))
        xin = ctx.enter_context(tc.tile_pool(name="xin", bufs=cfg["xin_bufs"]))
        rp = ctx.enter_context(tc.tile_pool(name="rp", bufs=cfg["r_bufs"]))
        sqp = ctx.enter_context(tc.tile_pool(name="sqp", bufs=cfg["sq_bufs"]))
        sm = ctx.enter_context(tc.tile_pool(name="sm", bufs=cfg["sm_bufs"]))
        ps_mm = ctx.enter_context(
            tc.tile_pool(name="ps_mm", bufs=cfg["ps_mm_bufs"], space="PSUM"))
        ps_st = ctx.enter_context(
            tc.tile_pool(name="ps_st", bufs=cfg["ps_st_bufs"], space="PSUM"))
        ps_p3 = ctx.enter_context(
            tc.tile_pool(name="ps_p3", bufs=cfg["ps_p3_bufs"], space="PSUM"))

        # ---- one-time constants
        w1a = const.tile([128, HID], F16, tag="w1a")
        nc.sync.dma_start(w1a[:], w1.ap()[0:128, :])
        w1b = const.tile([128, HID], F16, tag="w1b")
        nc.sync.dma_start(w1b[:], w1.ap()[128:256, :])
        w2r = []
        for k in range(MT):
            t = const.tile([128, HID], F16, name=f"w2r{k}", tag=f"w2r{k}")
            nc.sync.dma_start(t[:], w2.ap()[k * 128:(k + 1) * 128, :])
            w2r.append(t)
        w3r = const.tile([128, MT], F16, tag="w3r")
        nc.sync.dma_start(w3r[:], w3p.ap())
        s1 = const.tile([128, MT], F32, tag="s1")
        nc.sync.dma_start(s1[:], s1p.ap())
        s2 = const.tile([128, MT], F32, tag="s2")
        nc.sync.dma_start(s2[:], s2p.ap())
        b3s = const.tile([1, 1], F32, tag="b3s")
        nc.sync.dma_start(b3s[:], b3t.ap())
        ones_st = const.tile([128, 2, 32], F32, tag="ones_st")
        nc.vector.memset(ones_st[:], 1.0)
        ones8 = const.tile([128, 2, 32], F8, tag="ones8")
        nc.vector.tensor_copy(ones8[:], ones_st[:])
        eps_t = const.tile([1, 1], F32, tag="eps_t")
        nc.vector.memset(eps_t[:], EPS)

        r_split = cfg.get("r_split", False)

        def layer_block(G_pool, w_tiles, rhs_list, sq_s, out_tag):
            """Emit MT m-tiles: matmuls + Square->fp8 pairs + relu->f16.
            Returns (R slices list of [128, CH] f16, sq pair tiles list)."""
            if r_split:
                Rs = [rp.tile([128, CH], F16, name=f"{out_tag}_{m}",
                              tag=f"{out_tag}_{m}") for m in range(MT)]
                rsl = [t[:] for t in Rs]
            else:
                R = rp.tile([128, MT * CH], F16, name=out_tag, tag=out_tag)
                rsl = [R[:, m * CH:(m + 1) * CH] for m in range(MT)]
            sqs = []
            for pr in range(MT // 2):
                sq = sqp.tile([128, 2, CH], F8, name=f"{out_tag}sq{pr}",
                              tag=f"{out_tag}sq{pr}")
                sqs.append(sq)
            for m in range(MT):
                p = G_pool.tile([128, CH], F32, name="pmm", tag="pmm")
                nk = len(w_tiles)
                for k in range(nk):
                    nc.tensor.matmul(p[:], w_tiles[k][:, m * 128:(m + 1) * 128],
                                     rhs_list[k], start=(k == 0),
                                     stop=(k == nk - 1))
                nc.scalar.activation(sqs[m // 2][:, m % 2, :], p[:], AF.Square,
                                     scale=sq_s[:, m:m + 1])
                nc.vector.tensor_scalar(out=rsl[m], in0=p[:], scalar1=0.0,
                                        scalar2=None, op0=ALU.max)
            return rsl, sqs

        def stats_block(sqs, tag):
            from contextlib import nullcontext
            off = cfg.get("stats_prio_off", 0)
            stw = ps_st.tile([32, CH], F32, name=f"stw{tag}", tag="stw")
            with (tc.high_priority(offset=off) if off else nullcontext()):
                for i, sq in enumerate(sqs):
                    nc.tensor.matmul(stw[:], ones8[:], sq[:], start=(i == 0),
                                     stop=(i == len(sqs) - 1), perf_mode=DRM)
            return stw

        def chunk_partA(cs):
            """DMA + layer 1 + LN1 elementwise (PE work available early)."""
            xa = xin.tile([128, CH], F16, name="xa", tag="xa")
            nc.sync.dma_start(xa[:], xt.ap()[0:128, cs])
            xb = xin.tile([128, CH], F16, name="xb", tag="xb")
            nc.sync.dma_start(xb[:], xt.ap()[128:256, cs])
            R1, sq1 = layer_block(ps_mm, [w1a, w1b], [xa[:], xb[:]], s1, "R1")
            return R1, sq1

        def chunk_partB(cs, R1, sq1):
            """stats1, layer 2, LN2, layer 3, output."""
            st1 = stats_block(sq1, "1")
            pv1 = sm.tile([1, CH], F32, name="pv1", tag="pv1")
            nc.scalar.activation(pv1[:], st1[0:1, :], AF.Abs_reciprocal_sqrt,
                                 bias=eps_t[:], scale=1.0 / HID)

            R2, sq2 = layer_block(ps_mm, w2r, list(R1), s2, "R2")
            st2 = stats_block(sq2, "2")
            t1 = sm.tile([1, CH], F32, name="t1", tag="t1")
            nc.vector.tensor_mul(t1[:], pv1[:], pv1[:])
            u1 = sm.tile([1, CH], F32, name="u1", tag="u1")
            nc.vector.tensor_mul(u1[:], t1[:], st2[0:1, :])
            pv2 = sm.tile([1, CH], F32, name="pv2", tag="pv2")
            nc.scalar.activation(pv2[:], u1[:], AF.Abs_reciprocal_sqrt,
                                 bias=eps_t[:], scale=1.0 / HID)
            q2 = sm.tile([1, CH], F32, name="q2", tag="q2")
            nc.vector.tensor_mul(q2[:], pv1[:], pv2[:])

            p3 = ps_p3.tile([1, CH], F32, name="p3", tag="p3")
            for k in range(MT):
                nc.tensor.matmul(p3[:], w3r[:, k:k + 1], R2[k],
                                 start=(k == 0), stop=(k == MT - 1))
            osb = sm.tile([1, CH], F32, name="osb", tag="osb")
            nc.vector.tensor_mul(osb[:], p3[:], q2[:])
            if has_b3:
                b3b = bass.AP(tensor=b3s[:].tensor, offset=b3s[:].offset,
                              ap=[b3s[:].ap[0], [0, CH]])
                nc.vector.tensor_tensor(out=osb[:], in0=osb[:], in1=b3b,
                                        op=ALU.add)
            nc.sync.dma_start(y.ap()[0:1, cs], osb[:])

        def _cs(c):
            return slice(c * CH, (c + 1) * CH)

        def whole_body():
            cl = cfg.get("chunk_loop")
            if cl is not None:
                unroll = cfg.get("chunk_unroll", 1)
                hint = ((mybir.EngineType.PE,)
                        if cfg.get("hint_pe", False) else ())
                stag = cfg.get("staggered_reset", False)
                with tc.For_i(0, NCH // unroll, 1, hint_engines=hint,
                              staggered_reset=stag) as iv:
                    for u in range(unroll):
                        cs = bass.ds(iv * (CH * unroll) + u * CH, CH)
                        chunk_partB(cs, *chunk_partA(cs))
            elif cfg.get("skew", True):
                depth = cfg.get("skew_depth", 1)
                pend = [chunk_partA(_cs(c)) for c in range(min(depth, NCH))]
                for c in range(NCH):
                    if c + depth < NCH:
                        pend.append(chunk_partA(_cs(c + depth)))
                    chunk_partB(_cs(c), *pend.pop(0))
            else:
                for c in range(NCH):
                    chunk_partB(_cs(c), *chunk_partA(_cs(c)))

        if loop_iters is None:
            whole_body()
        else:
            with tc.For_i(0, loop_iters, 1):
                whole_body()
    nc.compile()
    return nc


def make_fast_maps(inp):
    """Host prep for the fast path. Returns (in_maps, has_b3)."""
    W1c, bc1, W2c, bc2 = _fold_weights(inp)
    g1 = np.asarray(inp["g1"], np.float64)
    g2 = np.asarray(inp["g2"], np.float64)
    W3 = np.asarray(inp["W3"], np.float32)
    b3 = np.asarray(inp["b3"], np.float32)

    XT = _build_xt_raw(inp)
    Xc = XT[128:185]
    XT[128:185] = np.sign(Xc) * np.log(np.abs(Xc) + 1.0)
    XTF = np.zeros((256, XT.shape[1]), np.float16)
    XTF[0:185] = XT.astype(np.float16)
    XTF[185] = 1.0

    W1g = (W1c.astype(np.float64) * g1[None, :])
    bc1g = bc1.astype(np.float64) * g1
    W1full = np.zeros((256, HID), np.float16)
    W1full[0:185] = W1g.astype(np.float16)
    W1full[185] = bc1g.astype(np.float16)
    W2g = (W2c.astype(np.float64) * g2[None, :]).astype(np.float16)

    shared = {
        "w1": W1full, "w2": W2g,
        "w3p": _pack128(W3[:, 0]).astype(np.float16),
        "s1p": _pack128(1.0 / g1), "s2p": _pack128(1.0 / g2),
        "b3t": b3.reshape(1, 1).astype(np.float32),
    }
    in_maps = []
    for c in range(N_CORES):
        m = dict(shared)
        m["xt"] = np.ascontiguousarray(XTF[:, c * B_CORE:(c + 1) * B_CORE])
        in_maps.append(m)
    return in_maps, bool(np.any(b3 != 0.0))


def fast_path_ok(inp):
    be1 = np.asarray(inp["be1"]); be2 = np.asarray(inp["be2"])
    g1 = np.asarray(inp["g1"]); g2 = np.asarray(inp["g2"])
    _, _, _, bc2 = _fold_weights(inp)
    return (np.all(be1 == 0.0) and np.all(be2 == 0.0)
            and np.all(np.abs(bc2) < 1e-12)
            and np.all(np.abs(g1) > 1e-6) and np.all(np.abs(g2) > 1e-6))


# ---------------------------------------------------------------- legacy path
DEFAULT_CFG = dict(h_bufs=1, sq_bufs=1, r1_bufs=1, r2_bufs=1,
                   ps_mm_bufs=3, xin_bufs=3, xr_bufs=2, per_m=False,
                   l2_fp16=False, h_fp16=False)


def build_program(simple_affine, loop_iters=None, cfg=None):
    """Legacy general-path program (arbitrary affine/bias)."""
    cfg = {**DEFAULT_CFG, **(cfg or {})}
    nc = bacc.Bacc("TRN2", target_bir_lowering=False, debug=False)
    xt = nc.dram_tensor("xt", [KA + KC, B_CORE], F32, kind="ExternalInput")
    w1 = nc.dram_tensor("w1", [KA + KC, HID], F32, kind="ExternalInput")
    w2 = nc.dram_tensor("w2", [HID, HID], F32, kind="ExternalInput")
    w3p = nc.dram_tensor("w3p", [128, MT], F32, kind="ExternalInput")
    bc1p = nc.dram_tensor("bc1p", [128, MT], F32, kind="ExternalInput")
    bc2p = nc.dram_tensor("bc2p", [128, MT], F32, kind="ExternalInput")
    g1p = nc.dram_tensor("g1p", [128, MT], F32, kind="ExternalInput")
    be1p = nc.dram_tensor("be1p", [128, MT], F32, kind="ExternalInput")
    g2p = nc.dram_tensor("g2p", [128, MT], F32, kind="ExternalInput")
    be2p = nc.dram_tensor("be2p", [128, MT], F32, kind="ExternalInput")
    b3t = nc.dram_tensor("b3t", [1, 1], F32, kind="ExternalInput")
    y = nc.dram_tensor("y", [1, B_CORE], F32, kind="ExternalOutput")

    from contextlib import ExitStack
    with tile.TileContext(nc) as tc, ExitStack() as ctx, \
            nc.allow_low_precision(reason="f32r rounding is intentional"):
        const = ctx.enter_context(tc.tile_pool(name="const", bufs=1))
        wstage = ctx.enter_context(tc.tile_pool(name="wstage", bufs=2))
        xin = ctx.enter_context(tc.tile_pool(name="xin", bufs=cfg["xin_bufs"]))
        xr = ctx.enter_context(tc.tile_pool(name="xr", bufs=cfg["xr_bufs"]))
        bigH = ctx.enter_context(tc.tile_pool(name="bigH", bufs=cfg["h_bufs"]))
        bigS = ctx.enter_context(tc.tile_pool(name="bigS", bufs=cfg["sq_bufs"]))
        bigR1 = ctx.enter_context(tc.tile_pool(name="bigR1", bufs=cfg["r1_bufs"]))
        bigR2 = ctx.enter_context(tc.tile_pool(name="bigR2", bufs=cfg["r2_bufs"]))
        small = ctx.enter_context(tc.tile_pool(name="small", bufs=cfg.get("small_bufs", 2)))
        ps_mm = ctx.enter_context(tc.tile_pool(name="ps_mm", bufs=cfg["ps_mm_bufs"], space="PSUM"))
        ps_st = ctx.enter_context(tc.tile_pool(name="ps_st", bufs=cfg.get("ps_st_bufs", 2), space="PSUM"))
        ps_vec = ctx.enter_context(tc.tile_pool(name="ps_vec", bufs=cfg.get("ps_vec_bufs", 2), space="PSUM"))

        w1a_r = const.tile([128, HID], F32R, tag="w1a")
        st = wstage.tile([128, HID], F32, tag="stage")
        nc.sync.dma_start(st[:], w1.ap()[0:128, :])
        nc.vector.tensor_copy(w1a_r[:], st[:])
        w1c_r = const.tile([KC, HID], F32R, tag="w1c")
        stc = wstage.tile([KC, HID], F32, tag="stagec")
        nc.sync.dma_start(stc[:], w1.ap()[128:185, :])
        nc.vector.tensor_copy(w1c_r[:], stc[:])
        L2DT = mybir.dt.float16 if cfg["l2_fp16"] else F32R
        w2r = []
        for k in range(MT):
            stk = wstage.tile([128, HID], F32, tag="stage")
            nc.sync.dma_start(stk[:], w2.ap()[k * 128:(k + 1) * 128, :])
            t = const.tile([128, HID], L2DT, tag=f"w2r{k}")
            nc.vector.tensor_copy(t[:], stk[:])
            w2r.append(t)
        w3p_r = const.tile([128, MT], L2DT, tag="w3p")
        st3 = wstage.tile([128, MT], F32, tag="stages")
        nc.sync.dma_start(st3[:], w3p.ap())
        nc.vector.tensor_copy(w3p_r[:], st3[:])

        def load_small(name, dram):
            t = const.tile([128, MT], F32, tag=name)
            nc.sync.dma_start(t[:], dram.ap())
            return t
        bc1s = load_small("bc1s", bc1p); bc2s = load_small("bc2s", bc2p)
        g1s = load_small("g1s", g1p); be1s = load_small("be1s", be1p)
        g2s = load_small("g2s", g2p); be2s = load_small("be2s", be2p)
        b3s = const.tile([1, 1], F32, tag="b3s")
        nc.sync.dma_start(b3s[:], b3t.ap())
        ones_st = const.tile([128, 1], F32, tag="ones_st")
        nc.vector.memset(ones_st[:], 1.0)
        ones_col = const.tile([128, 1], F32R, tag="ones_col")
        nc.vector.tensor_copy(ones_col[:], ones_st[:])
        ones_rst = const.tile([1, 128], F32, tag="ones_rst")
        nc.vector.memset(ones_rst[:], 1.0)
        ones_row = const.tile([1, 128], F32R, tag="ones_row")
        nc.vector.tensor_copy(ones_row[:], ones_rst[:])
        eps_t = const.tile([1, 1], F32, tag="eps_t")
        nc.vector.memset(eps_t[:], EPS)

        def layer_norm_relu(Hb, g_s, be_s, out_pool, out_tag):
            sqb = bigS.tile([128, MT * CH], F32R, tag="sq")
            if cfg["per_m"]:
                for m in range(MT):
                    sl = slice(m * CH, (m + 1) * CH)
                    nc.vector.tensor_mul(sqb[:, sl], Hb[:, sl], Hb[:, sl])
            else:
                nc.vector.tensor_mul(sqb[:], Hb[:], Hb[:])
            pst = ps_st.tile([1, CH], F32, tag="pst")
            for m in range(MT):
                nc.tensor.matmul(pst[:], ones_col[:],
                                 sqb[:, m * CH:(m + 1) * CH],
                                 start=(m == 0), stop=(m == MT - 1))
            sd = small.tile([1, CH], F32, tag="sd")
            nc.scalar.activation(sd[:], pst[:], AF.Sqrt,
                                 bias=eps_t[:], scale=1.0 / HID)
            rs = small.tile([1, CH], F32R, tag="rs")
            nc.vector.reciprocal(rs[:], sd[:])
            pv = ps_vec.tile([128, CH], F32, tag="pv")
            nc.tensor.matmul(pv[:], ones_row[:], rs[:], start=True, stop=True)
            Rb = out_pool.tile([128, MT * CH], L2DT, tag=out_tag)
            if cfg["per_m"]:
                for m in range(MT):
                    sl = slice(m * CH, (m + 1) * CH)
                    nc.vector.tensor_mul(Hb[:, sl], Hb[:, sl], pv[:])
                    if simple_affine:
                        nc.scalar.activation(Rb[:, sl], Hb[:, sl], AF.Relu)
                    else:
                        nc.scalar.activation(Rb[:, sl], Hb[:, sl], AF.Relu,
                                             bias=be_s[:, m:m + 1],
                                             scale=g_s[:, m:m + 1])
            else:
                h3 = Hb[:].rearrange("p (m n) -> p m n", m=MT)
                pvb = bass.AP(tensor=pv[:].tensor, offset=pv[:].offset,
                              ap=[pv[:].ap[0], [0, MT], pv[:].ap[1]])
                nc.vector.tensor_mul(h3, h3, pvb)
                if simple_affine:
                    nc.scalar.activation(Rb[:], Hb[:], AF.Relu)
                else:
                    for m in range(MT):
                        sl = slice(m * CH, (m + 1) * CH)
                        nc.scalar.activation(Rb[:, sl], Hb[:, sl], AF.Relu,
                                             bias=be_s[:, m:m + 1],
                                             scale=g_s[:, m:m + 1])
            return Rb

        HDT = mybir.dt.float16 if cfg["h_fp16"] else F32

        def chunk_body(c):
            x1 = xin.tile([128, CH], F32, tag="x1")
            nc.sync.dma_start(x1[:], xt.ap()[0:128, c * CH:(c + 1) * CH])
            x2 = xin.tile([KC, CH], F32, tag="x2")
            nc.sync.dma_start(x2[:], xt.ap()[128:185, c * CH:(c + 1) * CH])
            x1r = xr.tile([128, CH], F32R, tag="x1r")
            nc.vector.tensor_copy(x1r[:], x1[:])
            xab = xr.tile([KC, CH], F32, tag="xab")
            nc.vector.tensor_scalar(
                out=xab[:].bitcast(mybir.dt.int32),
                in0=x2[:].bitcast(mybir.dt.int32),
                scalar1=0x7FFFFFFF, scalar2=None, op0=ALU.bitwise_and)
            xln = xr.tile([KC, CH], F32, tag="xln")
            nc.scalar.activation(xln[:], xab[:], AF.Ln, bias=1.0)
            xsg = xr.tile([KC, CH], F32, tag="xsg")
            nc.scalar.activation(xsg[:], x2[:], AF.Sign)
            x2r = xr.tile([KC, CH], F32R, tag="x2r")
            nc.vector.tensor_mul(x2r[:], xsg[:], xln[:])

            H1 = bigH.tile([128, MT * CH], HDT, tag="H")
            for m in range(MT):
                p1 = ps_mm.tile([128, CH], F32, tag="pmm")
                nc.tensor.matmul(p1[:], w1a_r[:, m * 128:(m + 1) * 128],
                                 x1r[:], start=True, stop=False)
                nc.tensor.matmul(p1[:], w1c_r[:, m * 128:(m + 1) * 128],
                                 x2r[:], start=False, stop=True)
                nc.scalar.activation(H1[:, m * CH:(m + 1) * CH], p1[:],
                                     AF.Identity, bias=bc1s[:, m:m + 1])
            R1 = layer_norm_relu(H1, g1s, be1s, bigR1, "R1")

            H2 = bigH.tile([128, MT * CH], HDT, tag="H")
            for m in range(MT):
                p2 = ps_mm.tile([128, CH], F32, tag="pmm")
                for k in range(MT):
                    nc.tensor.matmul(p2[:], w2r[k][:, m * 128:(m + 1) * 128],
                                     R1[:, k * CH:(k + 1) * CH],
                                     start=(k == 0), stop=(k == MT - 1))
                nc.scalar.activation(H2[:, m * CH:(m + 1) * CH], p2[:],
                                     AF.Identity, bias=bc2s[:, m:m + 1])
            R2 = layer_norm_relu(H2, g2s, be2s, bigR2, "R2")

            p3 = ps_st.tile([1, CH], F32, tag="pst")
            for k in range(MT):
                nc.tensor.matmul(p3[:], w3p_r[:, k:k + 1],
                                 R2[:, k * CH:(k + 1) * CH],
                                 start=(k == 0), stop=(k == MT - 1))
            osb = small.tile([1, CH], F32, tag="osb")
            nc.scalar.activation(osb[:], p3[:], AF.Identity, bias=b3s[:])
            nc.sync.dma_start(y.ap()[0:1, c * CH:(c + 1) * CH], osb[:])

        if loop_iters is None:
            for c in range(NCH):
                chunk_body(c)
        else:
            with tc.For_i(0, loop_iters, 1):
                for c in range(NCH):
                    chunk_body(c)
    nc.compile()
    return nc


def _build_xt_legacy(inp):
    XT = np.zeros((KA + KC, inp["op_vec"].shape[0]), np.float32)
    XT[0:185] = _build_xt_raw(inp)
    return XT


def make_legacy_maps(inp):
    W1c, bc1, W2c, bc2 = _fold_weights(inp)
    XT = _build_xt_legacy(inp)
    g1 = np.asarray(inp["g1"], np.float32); be1 = np.asarray(inp["be1"], np.float32)
    g2 = np.asarray(inp["g2"], np.float32); be2 = np.asarray(inp["be2"], np.float32)
    simple_affine = bool(
        np.all(g1 == 1.0) and np.all(g2 == 1.0)
        and np.all(be1 == 0.0) and np.all(be2 == 0.0))
    W3 = np.asarray(inp["W3"], np.float32)
    b3 = np.asarray(inp["b3"], np.float32)
    shared = {
        "w1": W1c, "w2": W2c,
        "w3p": _pack128(W3[:, 0]),
        "bc1p": _pack128(bc1), "bc2p": _pack128(bc2),
        "g1p": _pack128(g1), "be1p": _pack128(be1),
        "g2p": _pack128(g2), "be2p": _pack128(be2),
        "b3t": b3.reshape(1, 1),
    }
    in_maps = []
    for c in range(N_CORES):
        m = dict(shared)
        m["xt"] = np.ascontiguousarray(XT[:, c * B_CORE:(c + 1) * B_CORE])
        in_maps.append(m)
    return in_maps, simple_affine


# ---------------------------------------------------------------- entry point
_CACHE = {}

BEST_CFG = dict(ps_mm_bufs=5, ps_st_bufs=2, ps_p3_bufs=1)
LEGACY_BEST_CFG = dict(per_m=True, h_bufs=2, ps_mm_bufs=4)


def make_in_maps(inputs):
    """Returns (in_maps, mode) with mode = ("fast", has_b3) or
    ("legacy", simple_affine)."""
    inp = {k: np.asarray(v) for k, v in inputs.items()}
    if fast_path_ok(inp):
        in_maps, has_b3 = make_fast_maps(inp)
        return in_maps, ("fast", has_b3)
    in_maps, simple_affine = make_legacy_maps(inp)
    return in_maps, ("legacy", simple_affine)


def build_for_mode(mode, loop_iters=None, cfg=None):
    kind, flag = mode
    if kind == "fast":
        return build_fast(flag, loop_iters=loop_iters,
                          cfg=cfg if cfg is not None else BEST_CFG)
    return build_program(flag, loop_iters=loop_iters,
                         cfg=cfg if cfg is not None else LEGACY_BEST_CFG)


def _get_program(mode):
    key = ("prog", mode)
    if key not in _CACHE:
        _CACHE[key] = build_for_mode(mode)
    return _CACHE[key]


def kernel(**inputs) -> np.ndarray:
    in_maps, mode = make_in_maps(inputs)
    nc = _get_program(mode)
    res = run_bass_kernel_spmd(nc, in_maps, core_ids=list(range(N_CORES)))
    y = np.concatenate([r["y"][0] for r in res.results])
    return y.reshape(B, 1).astype(np.float32)


if __name__ == "__main__":
    import jax
    import reference
    cpu = jax.devices("cpu")[0]
    with jax.default_device(cpu):
        inp = reference.setup_inputs()
        ref = np.asarray(reference.reference(**inp))
    out = kernel(**{k: np.asarray(v) for k, v in inp.items()})
    err = np.abs(out - ref)
    scale = np.abs(ref).max()
    print("max_abs", err.max(), "rel(vs scale)", err.max() / scale,
          "mean_rel", (err / (np.abs(ref) + 1e-6)).mean())

